# revision 19
# baseline (speedup 1.0000x reference)
"""Trainium2 Bass kernel for GQA attention block (B=2, S=2048, HID=4096, 32Q/8KV heads).

Sharding: hybrid TP4 x DP2 over 8 NeuronCores.
  core c: batch b = c // 4, TP slice t = c % 4.
  Each core handles one batch element, 8 Q heads (2 KV heads): q/k/v projection
  output dim sharded; o_proj input dim sharded -> partial outputs summed on host.

On-chip dataflow is feature-major ("transposed"): qT/kT [head_dim, tokens] so that
attention scores come out keys-major (softmax key-sum over partition via a
ones-matmul) and the attention output lands directly in the layout o_proj needs
as lhsT. RoPE's rotate_half is two SBUF->SBUF partition-shift DMAs with the sign
folded into a host-negated sin table; the combine runs on VectorE. PSUM
evacuations ride on ScalarE. dt="bf16" (default speed path) or "f32r"
(fp32-storage, ~1e-3 matmul precision, same PE rate at N>=256).
"""
import os
import sys

for _p in ("/opt/trn_rl_repo", "/root/.axon_site"):
    if _p not in sys.path and os.path.isdir(_p):
        sys.path.append(_p)

import numpy as np

B, S_FULL, HID = 2, 2048, 4096
NH, NKV, HD = 32, 8, 128
TP = 4                 # tensor-parallel ways
QH = NH // TP          # 8 q heads per core
KVH = NKV // TP        # 2 kv heads per core
FQ = QH * HD           # 1024
FKV = KVH * HD         # 256
KH = HID // 128        # 32 contraction tiles
SCALE = 1.0 / float(np.sqrt(HD))

last_exec_time_ns = None


def build_nc(S: int = S_FULL, dt: str = "bf16"):
    """Build the per-core Bass program (SPMD: same program, per-core inputs)."""
    import concourse.bass as bass
    import concourse.tile as tile
    from concourse import bacc, mybir
    from contextlib import ExitStack

    f32 = mybir.dt.float32
    mdt = mybir.dt.bfloat16 if dt == "bf16" else mybir.dt.float32r
    ddt = mdt if dt == "bf16" else f32   # DRAM dtype for big inputs
    TBP = min(1024, S)                   # projection token block (2 PSUM banks/tag)
    QW = min(1024, S)                    # attention query window (exp tile width)
    NQW = S // QW
    H5 = [slice(j * 512, min((j + 1) * 512, QW)) for j in range((QW + 511) // 512)]
    NTB = S // TBP
    NB5 = [slice(j * 512, min((j + 1) * 512, TBP)) for j in range((TBP + 511) // 512)]
    KT = S // 128                        # key/token tiles
    HALF = HD // 2

    nc = bacc.Bacc("TRN2", target_bir_lowering=False, debug=False)

    hsT = nc.dram_tensor("hsT", [HID, S], ddt, kind="ExternalInput")
    cosT = nc.dram_tensor("cosT", [HD, S], ddt, kind="ExternalInput")
    sinT = nc.dram_tensor("sinT", [HD, S], ddt, kind="ExternalInput")  # sign-folded
    wq = nc.dram_tensor("wq", [HID, FQ], ddt, kind="ExternalInput")
    bq = nc.dram_tensor("bq", [QH, HD], f32, kind="ExternalInput")
    wkv = nc.dram_tensor("wkv", [HID, 2 * FKV], ddt, kind="ExternalInput")
    bk = nc.dram_tensor("bk", [KVH, HD], f32, kind="ExternalInput")
    bv = nc.dram_tensor("bv", [KVH, HD], f32, kind="ExternalInput")
    wo = nc.dram_tensor("wo", [FQ, HID], ddt, kind="ExternalInput")
    ident = nc.dram_tensor("ident", [128, 128], ddt, kind="ExternalInput")
    ones = nc.dram_tensor("ones", [128, 128], ddt, kind="ExternalInput")
    out = nc.dram_tensor("out", [S, HID], f32, kind="ExternalOutput")

    def bc(ap):
        """View a DRAM fp32 AP as f32r (no-op for bf16)."""
        return ap.bitcast(mdt) if dt == "f32r" else ap

    with tile.TileContext(nc) as tc, ExitStack() as ctx:
        Exp = mybir.ActivationFunctionType.Exp
        Ident = mybir.ActivationFunctionType.Identity

        const = ctx.enter_context(tc.tile_pool(name="const", bufs=1))
        bq_t = const.tile([128, QH], f32)
        bk_t = const.tile([128, KVH], f32)
        bv_t = const.tile([128, KVH], f32)
        id_t = const.tile([128, 128], mdt)
        ones_t = const.tile([128, 128], mdt)
        cos_t = const.tile([128, S], mdt)
        sin_t = const.tile([128, S], mdt)

        # Persistent activations (feature-major). attnT overwrites q in place.
        qpool = ctx.enter_context(tc.tile_pool(name="qpool", bufs=1))
        q_t = [qpool.tile([128, S], mdt, name=f"q{h}") for h in range(QH)]
        kvpool = ctx.enter_context(tc.tile_pool(name="kvpool", bufs=1))
        k_t = [kvpool.tile([128, S], mdt, name=f"k{f}") for f in range(KVH)]
        v_t = kvpool.tile([128, KT * FKV], mdt, name="v")  # [tok%128, (kt, kv*128+d)]

        rope_pool = ctx.enter_context(tc.tile_pool(name="ropep", bufs=1))

        def rope_inplace(x_t, sl, tag):
            """x[:, sl] = x[:, sl]*cos + shift64(x[:, sl])*sin', all engines cheap:
            rotate_half = two SBUF->SBUF partition-shift DMAs (sign is in sin')."""
            w = sl.stop - sl.start
            rot = rope_pool.tile([128, 1024], mdt, name=f"rot_{tag}", tag="rot", bufs=6)
            nc.sync.dma_start(rot[0:HALF, :w], x_t[HALF:128, sl])
            nc.sync.dma_start(rot[HALF:128, :w], x_t[0:HALF, sl])
            t1 = rope_pool.tile([128, 1024], mdt, name=f"t1_{tag}", tag="t1", bufs=6)
            nc.vector.tensor_mul(t1[:, :w], rot[:, :w], sin_t[:, sl])
            nc.vector.tensor_mul(x_t[:, sl], x_t[:, sl], cos_t[:, sl])
            nc.vector.tensor_add(x_t[:, sl], x_t[:, sl], t1[:, :w])

        # ---- Phase 1: K/V projections (feature-major) ----
        with tc.tile_pool(name="vTp", bufs=1) as vTp:
            vT_t = [vTp.tile([128, S], mdt, name=f"vT{f}") for f in range(KVH)]
            with (
                tc.tile_pool(name="st1", bufs=8) as st1,
                tc.tile_pool(name="pkv", bufs=1, space="PSUM") as pkv,
            ):
                for tb in range(NTB):
                    tb0 = tb * TBP
                    psk = [[pkv.tile([128, sl.stop - sl.start], f32,
                                     name=f"psk_{tb}_{f}_{j}", tag=f"psk{f}_{j}")
                            for j, sl in enumerate(NB5)] for f in range(KVH)]
                    psv = [[pkv.tile([128, sl.stop - sl.start], f32,
                                     name=f"psv_{tb}_{f}_{j}", tag=f"psv{f}_{j}")
                            for j, sl in enumerate(NB5)] for f in range(KVH)]
                    for k in range(KH):
                        hs_s = st1.tile([128, TBP], mdt, name=f"hs_{tb}_{k}", tag="hs")
                        nc.sync.dma_start(
                            hs_s[:], bc(hsT.ap()[k * 128:(k + 1) * 128, tb0:tb0 + TBP]))
                        wkv_s = st1.tile([128, 2 * FKV], mdt, name=f"wkv_{tb}_{k}",
                                         tag="wkv", bufs=4)
                        nc.sync.dma_start(wkv_s[:], bc(wkv.ap()[k * 128:(k + 1) * 128, :]))
                        wk_s = wkv_s[:, 0:FKV]
                        wv_s = wkv_s[:, FKV:2 * FKV]
                        for f in range(KVH):
                            for j, sl in enumerate(NB5):
                                nc.tensor.matmul(psk[f][j][:], wk_s[:, f * 128:(f + 1) * 128],
                                                 hs_s[:, sl], start=(k == 0), stop=(k == KH - 1))
                        for f in range(KVH):
                            for j, sl in enumerate(NB5):
                                nc.tensor.matmul(psv[f][j][:], wv_s[:, f * 128:(f + 1) * 128],
                                                 hs_s[:, sl], start=(k == 0), stop=(k == KH - 1))
                    if tb == 0:
                        nc.sync.dma_start(bq_t[:], bq.ap().rearrange("h p -> p h"))
                        nc.sync.dma_start(bk_t[:], bk.ap().rearrange("h p -> p h"))
                        nc.sync.dma_start(bv_t[:], bv.ap().rearrange("h p -> p h"))
                        nc.sync.dma_start(id_t[:], bc(ident.ap()))
                        nc.sync.dma_start(ones_t[:], bc(ones.ap()))
                        nc.sync.dma_start(cos_t[:], bc(cosT.ap()))
                        nc.sync.dma_start(sin_t[:], bc(sinT.ap()))
                    for f in range(KVH):
                        for j, sl in enumerate(NB5):
                            tsl = slice(tb0 + sl.start, tb0 + sl.stop)
                            nc.scalar.activation(k_t[f][:, tsl], psk[f][j][:], Ident,
                                                 bias=bk_t[:, f:f + 1])
                            nc.vector.tensor_scalar_add(vT_t[f][:, tsl], psv[f][j][:],
                                                        bv_t[:, f:f + 1])
                    # RoPE on this tb's K tokens (PSUM-free; overlaps the next pass)
                    for f in range(KVH):
                        rope_inplace(k_t[f], slice(tb0, tb0 + TBP), f"k{f}_{tb}")

            # ---- Phase 2: transpose vT -> v (token-major) via PE, 4 per bank ----
            with tc.tile_pool(name="pr1", bufs=1, space="PSUM") as pr1:
                vv = v_t[:].rearrange("p (kt fkv) -> p kt fkv", fkv=FKV)
                GT = 4 if KT % 4 == 0 else 1
                for f in range(KVH):
                    for kt0 in range(0, KT, GT):
                        pst = pr1.tile([128, GT * 128], mdt, name=f"pst_{f}_{kt0}",
                                       tag="tp", bufs=2)
                        for j in range(GT):
                            nc.tensor.transpose(
                                pst[:, j * 128:(j + 1) * 128],
                                vT_t[f][:, (kt0 + j) * 128:(kt0 + j + 1) * 128], id_t[:])
                        nc.scalar.copy(
                            vv[:, kt0:kt0 + GT, f * 128:(f + 1) * 128],
                            pst[:].rearrange("p (j c) -> p j c", c=128))

        # ---- Phase 3: Q projection (feature-major) ----
        FC = max(1, QH // 4)  # chunks of up to 4 heads
        HPC = QH // FC
        with (
            tc.tile_pool(name="st2", bufs=8) as st2,
            tc.tile_pool(name="pq", bufs=1, space="PSUM") as pq,
        ):
            for fc in range(FC):
                for tb in range(NTB):
                    tb0 = tb * TBP
                    psq = [[pq.tile([128, sl.stop - sl.start], f32,
                                    name=f"psq_{fc}_{tb}_{i}_{j}", tag=f"psq{i}_{j}")
                            for j, sl in enumerate(NB5)] for i in range(HPC)]
                    for k in range(KH):
                        hs_s = st2.tile([128, TBP], mdt, name=f"hsq_{fc}_{tb}_{k}", tag="hs")
                        nc.sync.dma_start(
                            hs_s[:], bc(hsT.ap()[k * 128:(k + 1) * 128, tb0:tb0 + TBP]))
                        wq_s = st2.tile([128, HPC * 128], mdt, name=f"wq_{fc}_{tb}_{k}", tag="wq")
                        nc.sync.dma_start(
                            wq_s[:], bc(wq.ap()[k * 128:(k + 1) * 128,
                                                fc * HPC * 128:(fc + 1) * HPC * 128]))
                        for i in range(HPC):
                            for j, sl in enumerate(NB5):
                                nc.tensor.matmul(psq[i][j][:], wq_s[:, i * 128:(i + 1) * 128],
                                                 hs_s[:, sl], start=(k == 0), stop=(k == KH - 1))
                    for i in range(HPC):
                        h = fc * HPC + i
                        for j, sl in enumerate(NB5):
                            tsl = slice(tb0 + sl.start, tb0 + sl.stop)
                            if (i + j) % 2 == 0:
                                nc.scalar.activation(q_t[h][:, tsl], psq[i][j][:], Ident,
                                                     bias=bq_t[:, h:h + 1])
                            else:
                                nc.vector.tensor_scalar_add(q_t[h][:, tsl], psq[i][j][:],
                                                            bq_t[:, h:h + 1])
                # RoPE for completed heads of this chunk (overlaps next pass)
                if tb == NTB - 1:
                    wb = min(1024, S)
                    for i in range(HPC):
                        h = fc * HPC + i
                        for qb in range(S // wb):
                            rope_inplace(q_t[h], slice(qb * wb, (qb + 1) * wb), f"q{h}_{qb}")

        # ---- Phase 4: attention per (head, 512-wide query block) ----
        QB = min(512, S)
        NQB = S // QB
        st3 = ctx.enter_context(tc.tile_pool(name="st3", bufs=4))
        wo_pre = []
        for fh in range(QH):
            w = st3.tile([128, 512], mdt, name=f"wo_0_{fh}", tag=f"wo{fh}", bufs=2)
            nc.sync.dma_start(w[:], bc(wo.ap()[fh * 128:(fh + 1) * 128, 0:512]))
            wo_pre.append(w)
        with (
            tc.tile_pool(name="expp", bufs=2) as expp,
            tc.tile_pool(name="spool", bufs=2) as spool,
            tc.tile_pool(name="invp", bufs=3) as invp,
            tc.tile_pool(name="pss", bufs=1, space="PSUM") as pss,
            tc.tile_pool(name="pso", bufs=1, space="PSUM") as pso,
            tc.tile_pool(name="psb", bufs=1, space="PSUM") as psb,
        ):
            for h in range(QH):
                f = h // (QH // KVH)  # local kv head (GQA group of 4)
                for qb in range(NQB):
                    sl = slice(qb * QB, (qb + 1) * QB)
                    po = pso.tile([128, QB], f32, name=f"po_{h}_{qb}", tag="oo", bufs=1)
                    # kt-paired score tiles: one Exp per two key tiles; carry-tree sums
                    KP = 2 if KT % 2 == 0 else 1
                    ranks = {}
                    for kt0 in range(0, KT, KP):
                        ps = pss.tile([128, KP * QB], f32, name=f"ps_{h}_{qb}_{kt0}",
                                      tag="ss", bufs=3 if KP == 2 else 5)
                        for j in range(KP):
                            nc.tensor.matmul(ps[:, j * QB:(j + 1) * QB],
                                             k_t[f][:, (kt0 + j) * 128:(kt0 + j + 1) * 128],
                                             q_t[h][:, sl], start=True, stop=True)
                        et = expp.tile([128, KP * QB], mdt, name=f"e_{h}_{qb}_{kt0}",
                                       tag="et", bufs=10)
                        nc.scalar.activation(et[:], ps[:], Exp, scale=SCALE)
                        for j in range(KP):
                            kt = kt0 + j
                            nc.tensor.matmul(po[:], v_t[:, kt * FKV + f * 128: kt * FKV + (f + 1) * 128],
                                             et[:, j * QB:(j + 1) * QB],
                                             start=(kt == 0), stop=(kt == KT - 1))
                        if KP == 2:
                            node = spool.tile([128, QB], mdt, name=f"pa_{h}_{qb}_{kt0}",
                                              tag=f"pa{(kt0 // 2) % 4}", bufs=3)
                            nc.vector.tensor_add(node[:], et[:, 0:QB], et[:, QB:2 * QB])
                            rank = 1
                        else:
                            node, rank = et, 0
                        while rank in ranks:
                            prev = ranks.pop(rank)
                            nc.vector.tensor_add(prev[:], prev[:], node[:])
                            node, rank = prev, rank + 1
                        ranks[rank] = node
                    rem = [ranks[r] for r in sorted(ranks)]
                    ssum = rem[0]
                    for other in rem[1:]:
                        nc.vector.tensor_add(ssum[:], ssum[:], other[:])
                    pb = psb.tile([128, QB], f32, name=f"pb_{h}_{qb}", tag="bb", bufs=1)
                    nc.tensor.matmul(pb[:], ones_t[:], ssum[:], start=True, stop=True)
                    inv = invp.tile([128, QB], f32, name=f"inv_{h}_{qb}", tag="inv")
                    nc.vector.reciprocal_approx_fast(inv[:], pb[:])
                    # normalized attn output, overwrites q head in place
                    nc.vector.tensor_mul(q_t[h][:, sl], po[:], inv[:])

        # ---- Phase 5: output projection (partial; host sums over TP) ----
        with (
            tc.tile_pool(name="osb", bufs=4) as osb,
            tc.tile_pool(name="po5", bufs=1, space="PSUM") as po5,
        ):
            NHB = HID // 512
            for hb in range(NHB):
                if hb == 0:
                    wo_s = wo_pre
                else:
                    wo_s = []
                    for fh in range(QH):
                        w = st3.tile([128, 512], mdt, name=f"wo_{hb}_{fh}", tag=f"wo{fh}", bufs=2)
                        nc.sync.dma_start(
                            w[:], bc(wo.ap()[fh * 128:(fh + 1) * 128, hb * 512:(hb + 1) * 512]))
                        wo_s.append(w)
                for tt in range(KT):
                    pot = po5.tile([128, 512], f32, name=f"pot_{hb}_{tt}", tag="po", bufs=4)
                    for fh in range(QH):
                        nc.tensor.matmul(pot[:], q_t[fh][:, tt * 128:(tt + 1) * 128],
                                         wo_s[fh][:], start=(fh == 0), stop=(fh == QH - 1))
                    ot = osb.tile([128, 512], f32, name=f"ot_{hb}_{tt}", tag="ot")
                    nc.scalar.copy(ot[:], pot[:])
                    nc.sync.dma_start(
                        out.ap()[tt * 128:(tt + 1) * 128, hb * 512:(hb + 1) * 512], ot[:])

    nc.compile()
    return nc


def make_host_constants():
    ident = np.eye(128, dtype=np.float32)
    ones = np.ones((128, 128), dtype=np.float32)
    return ident, ones


def shard_inputs(hidden_states, cos, sin, Wq, bq, Wk, bk, Wv, bv, Wo, S=S_FULL,
                 dt="bf16"):
    ident, ones = make_host_constants()
    if dt == "bf16":
        import ml_dtypes
        big = ml_dtypes.bfloat16
    else:
        big = np.float32
    in_maps = []
    for c in range(8):
        b, t = c // TP, c % TP
        sinT = np.ascontiguousarray(sin[b].T).astype(np.float32)
        sinT[:HD // 2, :] *= -1.0   # rotate_half sign folded into the table
        m = {
            "hsT": np.ascontiguousarray(hidden_states[b].T).astype(big),
            "cosT": np.ascontiguousarray(cos[b].T).astype(big),
            "sinT": sinT.astype(big),
            "wq": np.ascontiguousarray(Wq[:, t * FQ:(t + 1) * FQ]).astype(big),
            "bq": np.ascontiguousarray(bq[t * FQ:(t + 1) * FQ].reshape(QH, HD)),
            "bk": np.ascontiguousarray(bk[t * FKV:(t + 1) * FKV].reshape(KVH, HD)),
            "bv": np.ascontiguousarray(bv[t * FKV:(t + 1) * FKV].reshape(KVH, HD)),
            "wkv": np.ascontiguousarray(np.concatenate(
                [Wk[:, t * FKV:(t + 1) * FKV], Wv[:, t * FKV:(t + 1) * FKV]],
                axis=1)).astype(big),
            "wo": np.ascontiguousarray(Wo[t * FQ:(t + 1) * FQ, :]).astype(big),
            "ident": ident.astype(big), "ones": ones.astype(big),
        }
        in_maps.append(m)
    return in_maps


_nc_cache = {}


def kernel(hidden_states, cos, sin, Wq, bq, Wk, bk, Wv, bv, Wo):
    global last_exec_time_ns
    from concourse.bass_utils import run_bass_kernel_spmd

    hidden_states = np.asarray(hidden_states, dtype=np.float32)
    cos = np.asarray(cos, dtype=np.float32)
    sin = np.asarray(sin, dtype=np.float32)
    S = hidden_states.shape[1]
    dt = os.environ.get("ATTN_DT", "bf16")
    if (S, dt) not in _nc_cache:
        _nc_cache[(S, dt)] = build_nc(S, dt)
    nc = _nc_cache[(S, dt)]
    in_maps = shard_inputs(hidden_states, cos, sin,
                           np.asarray(Wq, np.float32), np.asarray(bq, np.float32),
                           np.asarray(Wk, np.float32), np.asarray(bk, np.float32),
                           np.asarray(Wv, np.float32), np.asarray(bv, np.float32),
                           np.asarray(Wo, np.float32), S=S, dt=dt)
    trace = bool(int(os.environ.get("ATTN_TRACE", "0")))
    r = run_bass_kernel_spmd(nc, in_maps, list(range(8)), trace=trace)
    last_exec_time_ns = r.exec_time_ns
    outs = [r.results[c]["out"] for c in range(8)]
    full = np.empty((B, S, HID), dtype=np.float32)
    for b in range(B):
        full[b] = outs[b * TP]
        for t in range(1, TP):
            full[b] += outs[b * TP + t]
    return full


# revision 21
# speedup vs baseline: 1.0237x; 1.0237x over previous
"""Trainium2 Bass kernel for GQA attention block (B=2, S=2048, HID=4096, 32Q/8KV heads).

Sharding: hybrid TP4 x DP2 over 8 NeuronCores.
  core c: batch b = c // 4, TP slice t = c % 4.
  Each core handles one batch element, 8 Q heads (2 KV heads): q/k/v projection
  output dim sharded; o_proj input dim sharded -> partial outputs summed on host.

On-chip dataflow is feature-major ("transposed"): qT/kT [head_dim, tokens] so that
attention scores come out keys-major (softmax key-sum over partition via a
ones-matmul) and the attention output lands directly in the layout o_proj needs
as lhsT. RoPE's rotate_half is two SBUF->SBUF partition-shift DMAs with the sign
folded into a host-negated sin table; the combine runs on VectorE. PSUM
evacuations ride on ScalarE. dt="bf16" (default speed path) or "f32r"
(fp32-storage, ~1e-3 matmul precision, same PE rate at N>=256).
"""
import os
import sys

for _p in ("/opt/trn_rl_repo", "/root/.axon_site"):
    if _p not in sys.path and os.path.isdir(_p):
        sys.path.append(_p)

import numpy as np

B, S_FULL, HID = 2, 2048, 4096
NH, NKV, HD = 32, 8, 128
TP = 4                 # tensor-parallel ways
QH = NH // TP          # 8 q heads per core
KVH = NKV // TP        # 2 kv heads per core
FQ = QH * HD           # 1024
FKV = KVH * HD         # 256
KH = HID // 128        # 32 contraction tiles
SCALE = 1.0 / float(np.sqrt(HD))

last_exec_time_ns = None


def build_nc(S: int = S_FULL, dt: str = "bf16"):
    """Build the per-core Bass program (SPMD: same program, per-core inputs)."""
    import concourse.bass as bass
    import concourse.tile as tile
    from concourse import bacc, mybir
    from contextlib import ExitStack

    f32 = mybir.dt.float32
    mdt = mybir.dt.bfloat16 if dt == "bf16" else mybir.dt.float32r
    ddt = mdt if dt == "bf16" else f32   # DRAM dtype for big inputs
    TBP = min(1024, S)                   # projection token block (2 PSUM banks/tag)
    QW = min(1024, S)                    # attention query window (exp tile width)
    NQW = S // QW
    H5 = [slice(j * 512, min((j + 1) * 512, QW)) for j in range((QW + 511) // 512)]
    NTB = S // TBP
    NB5 = [slice(j * 512, min((j + 1) * 512, TBP)) for j in range((TBP + 511) // 512)]
    KT = S // 128                        # key/token tiles
    HALF = HD // 2

    nc = bacc.Bacc("TRN2", target_bir_lowering=False, debug=False)

    hsT = nc.dram_tensor("hsT", [HID, S], ddt, kind="ExternalInput")
    cosT = nc.dram_tensor("cosT", [HD, S], ddt, kind="ExternalInput")
    sinT = nc.dram_tensor("sinT", [HD, S], ddt, kind="ExternalInput")  # sign-folded
    wq = nc.dram_tensor("wq", [HID, FQ], ddt, kind="ExternalInput")
    bq = nc.dram_tensor("bq", [QH, HD], f32, kind="ExternalInput")
    wkv = nc.dram_tensor("wkv", [HID, 2 * FKV], ddt, kind="ExternalInput")
    bk = nc.dram_tensor("bk", [KVH, HD], f32, kind="ExternalInput")
    bv = nc.dram_tensor("bv", [KVH, HD], f32, kind="ExternalInput")
    wo = nc.dram_tensor("wo", [FQ, HID], ddt, kind="ExternalInput")
    ident = nc.dram_tensor("ident", [128, 128], ddt, kind="ExternalInput")
    ones = nc.dram_tensor("ones", [128, 128], ddt, kind="ExternalInput")
    out = nc.dram_tensor("out", [S, HID], f32, kind="ExternalOutput")

    def bc(ap):
        """View a DRAM fp32 AP as f32r (no-op for bf16)."""
        return ap.bitcast(mdt) if dt == "f32r" else ap

    with tile.TileContext(nc) as tc, ExitStack() as ctx:
        Exp = mybir.ActivationFunctionType.Exp
        Ident = mybir.ActivationFunctionType.Identity

        const = ctx.enter_context(tc.tile_pool(name="const", bufs=1))
        bq_t = const.tile([128, QH], f32)
        bk_t = const.tile([128, KVH], f32)
        bv_t = const.tile([128, KVH], f32)
        id_t = const.tile([128, 128], mdt)
        ones_t = const.tile([128, 128], mdt)
        cos_t = const.tile([128, S], mdt)
        sin_t = const.tile([128, S], mdt)

        # Persistent activations (feature-major). attnT overwrites q in place.
        qpool = ctx.enter_context(tc.tile_pool(name="qpool", bufs=1))
        q_t = [qpool.tile([128, S], mdt, name=f"q{h}") for h in range(QH)]
        kvpool = ctx.enter_context(tc.tile_pool(name="kvpool", bufs=1))
        k_t = [kvpool.tile([128, S], mdt, name=f"k{f}") for f in range(KVH)]
        v_t = kvpool.tile([128, KT * FKV], mdt, name="v")  # [tok%128, (kt, kv*128+d)]

        rope_pool = ctx.enter_context(tc.tile_pool(name="ropep", bufs=1))

        def rope_inplace(x_t, sl, tag):
            """x[:, sl] = x[:, sl]*cos + shift64(x[:, sl])*sin', all engines cheap:
            rotate_half = two SBUF->SBUF partition-shift DMAs (sign is in sin')."""
            w = sl.stop - sl.start
            rot = rope_pool.tile([128, 1024], mdt, name=f"rot_{tag}", tag="rot", bufs=6)
            nc.sync.dma_start(rot[0:HALF, :w], x_t[HALF:128, sl])
            nc.sync.dma_start(rot[HALF:128, :w], x_t[0:HALF, sl])
            t1 = rope_pool.tile([128, 1024], mdt, name=f"t1_{tag}", tag="t1", bufs=6)
            nc.vector.tensor_mul(t1[:, :w], rot[:, :w], sin_t[:, sl])
            nc.vector.tensor_mul(x_t[:, sl], x_t[:, sl], cos_t[:, sl])
            nc.vector.tensor_add(x_t[:, sl], x_t[:, sl], t1[:, :w])

        # ---- Phase 1: K/V projections (feature-major) ----
        with tc.tile_pool(name="vTp", bufs=1) as vTp:
            vT_t = [vTp.tile([128, S], mdt, name=f"vT{f}") for f in range(KVH)]
            with (
                tc.tile_pool(name="st1", bufs=8) as st1,
                tc.tile_pool(name="pkv", bufs=1, space="PSUM") as pkv,
            ):
                for tb in range(NTB):
                    tb0 = tb * TBP
                    psk = [[pkv.tile([128, sl.stop - sl.start], f32,
                                     name=f"psk_{tb}_{f}_{j}", tag=f"psk{f}_{j}")
                            for j, sl in enumerate(NB5)] for f in range(KVH)]
                    psv = [[pkv.tile([128, sl.stop - sl.start], f32,
                                     name=f"psv_{tb}_{f}_{j}", tag=f"psv{f}_{j}")
                            for j, sl in enumerate(NB5)] for f in range(KVH)]
                    for k in range(KH):
                        hs_s = st1.tile([128, TBP], mdt, name=f"hs_{tb}_{k}", tag="hs")
                        nc.sync.dma_start(
                            hs_s[:], bc(hsT.ap()[k * 128:(k + 1) * 128, tb0:tb0 + TBP]))
                        wkv_s = st1.tile([128, 2 * FKV], mdt, name=f"wkv_{tb}_{k}",
                                         tag="wkv", bufs=4)
                        nc.sync.dma_start(wkv_s[:], bc(wkv.ap()[k * 128:(k + 1) * 128, :]))
                        wk_s = wkv_s[:, 0:FKV]
                        wv_s = wkv_s[:, FKV:2 * FKV]
                        for f in range(KVH):
                            for j, sl in enumerate(NB5):
                                nc.tensor.matmul(psk[f][j][:], wk_s[:, f * 128:(f + 1) * 128],
                                                 hs_s[:, sl], start=(k == 0), stop=(k == KH - 1))
                        for f in range(KVH):
                            for j, sl in enumerate(NB5):
                                nc.tensor.matmul(psv[f][j][:], wv_s[:, f * 128:(f + 1) * 128],
                                                 hs_s[:, sl], start=(k == 0), stop=(k == KH - 1))
                    if tb == 0:
                        nc.sync.dma_start(bq_t[:], bq.ap().rearrange("h p -> p h"))
                        nc.sync.dma_start(bk_t[:], bk.ap().rearrange("h p -> p h"))
                        nc.sync.dma_start(bv_t[:], bv.ap().rearrange("h p -> p h"))
                        nc.sync.dma_start(id_t[:], bc(ident.ap()))
                        nc.sync.dma_start(ones_t[:], bc(ones.ap()))
                        nc.sync.dma_start(cos_t[:], bc(cosT.ap()))
                        nc.sync.dma_start(sin_t[:], bc(sinT.ap()))
                    for f in range(KVH):
                        for j, sl in enumerate(NB5):
                            tsl = slice(tb0 + sl.start, tb0 + sl.stop)
                            nc.scalar.activation(k_t[f][:, tsl], psk[f][j][:], Ident,
                                                 bias=bk_t[:, f:f + 1])
                            nc.vector.tensor_scalar_add(vT_t[f][:, tsl], psv[f][j][:],
                                                        bv_t[:, f:f + 1])
                    # RoPE on this tb's K tokens (PSUM-free; overlaps the next pass)
                    for f in range(KVH):
                        rope_inplace(k_t[f], slice(tb0, tb0 + TBP), f"k{f}_{tb}")

            # ---- Phase 2: transpose vT -> v (token-major) via PE, 4 per bank ----
            with tc.tile_pool(name="pr1", bufs=1, space="PSUM") as pr1:
                vv = v_t[:].rearrange("p (kt fkv) -> p kt fkv", fkv=FKV)
                GT = 4 if KT % 4 == 0 else 1
                for f in range(KVH):
                    for kt0 in range(0, KT, GT):
                        pst = pr1.tile([128, GT * 128], mdt, name=f"pst_{f}_{kt0}",
                                       tag="tp", bufs=2)
                        for j in range(GT):
                            nc.tensor.transpose(
                                pst[:, j * 128:(j + 1) * 128],
                                vT_t[f][:, (kt0 + j) * 128:(kt0 + j + 1) * 128], id_t[:])
                        nc.scalar.copy(
                            vv[:, kt0:kt0 + GT, f * 128:(f + 1) * 128],
                            pst[:].rearrange("p (j c) -> p j c", c=128))

        # ---- Phase 3: Q projection (feature-major) ----
        FC = max(1, QH // 4)  # chunks of up to 4 heads
        HPC = QH // FC
        with (
            tc.tile_pool(name="st2", bufs=8) as st2,
            tc.tile_pool(name="pq", bufs=1, space="PSUM") as pq,
        ):
            for fc in range(FC):
                for tb in range(NTB):
                    tb0 = tb * TBP
                    psq = [[pq.tile([128, sl.stop - sl.start], f32,
                                    name=f"psq_{fc}_{tb}_{i}_{j}", tag=f"psq{i}_{j}")
                            for j, sl in enumerate(NB5)] for i in range(HPC)]
                    for k in range(KH):
                        hs_s = st2.tile([128, TBP], mdt, name=f"hsq_{fc}_{tb}_{k}", tag="hs")
                        nc.sync.dma_start(
                            hs_s[:], bc(hsT.ap()[k * 128:(k + 1) * 128, tb0:tb0 + TBP]))
                        wq_s = st2.tile([128, HPC * 128], mdt, name=f"wq_{fc}_{tb}_{k}", tag="wq")
                        nc.sync.dma_start(
                            wq_s[:], bc(wq.ap()[k * 128:(k + 1) * 128,
                                                fc * HPC * 128:(fc + 1) * HPC * 128]))
                        for i in range(HPC):
                            for j, sl in enumerate(NB5):
                                nc.tensor.matmul(psq[i][j][:], wq_s[:, i * 128:(i + 1) * 128],
                                                 hs_s[:, sl], start=(k == 0), stop=(k == KH - 1))
                    for i in range(HPC):
                        h = fc * HPC + i
                        for j, sl in enumerate(NB5):
                            tsl = slice(tb0 + sl.start, tb0 + sl.stop)
                            if (i + j) % 2 == 0:
                                nc.scalar.activation(q_t[h][:, tsl], psq[i][j][:], Ident,
                                                     bias=bq_t[:, h:h + 1])
                            else:
                                nc.vector.tensor_scalar_add(q_t[h][:, tsl], psq[i][j][:],
                                                            bq_t[:, h:h + 1])
                # RoPE for completed heads of this chunk (overlaps next pass)
                if tb == NTB - 1:
                    wb = min(1024, S)
                    for i in range(HPC):
                        h = fc * HPC + i
                        for qb in range(S // wb):
                            rope_inplace(q_t[h], slice(qb * wb, (qb + 1) * wb), f"q{h}_{qb}")

        # ---- Phase 4: attention per (head, 512-wide query block) ----
        QB = min(512, S)
        NQB = S // QB
        st3 = ctx.enter_context(tc.tile_pool(name="st3", bufs=4))
        wo_pre = []
        for fh in range(QH):
            w = st3.tile([128, 512], mdt, name=f"wo_0_{fh}", tag=f"wo{fh}", bufs=2)
            nc.sync.dma_start(w[:], bc(wo.ap()[fh * 128:(fh + 1) * 128, 0:512]))
            wo_pre.append(w)
        with (
            tc.tile_pool(name="expp", bufs=2) as expp,
            tc.tile_pool(name="spool", bufs=2) as spool,
            tc.tile_pool(name="invp", bufs=2) as invp,
            tc.tile_pool(name="pss", bufs=1, space="PSUM") as pss,
            tc.tile_pool(name="pso", bufs=1, space="PSUM") as pso,
            tc.tile_pool(name="psb", bufs=1, space="PSUM") as psb,
        ):
            for h in range(QH):
                f = h // (QH // KVH)  # local kv head (GQA group of 4)
                for qb in range(NQB):
                    sl = slice(qb * QB, (qb + 1) * QB)
                    po = pso.tile([128, QB], f32, name=f"po_{h}_{qb}", tag="oo", bufs=2)
                    # kt-paired score tiles: one Exp per two key tiles; carry-tree sums
                    KP = 2 if KT % 2 == 0 else 1
                    ranks = {}
                    for kt0 in range(0, KT, KP):
                        ps = pss.tile([128, KP * QB], f32, name=f"ps_{h}_{qb}_{kt0}",
                                      tag="ss", bufs=2 if KP == 2 else 5)
                        for j in range(KP):
                            nc.tensor.matmul(ps[:, j * QB:(j + 1) * QB],
                                             k_t[f][:, (kt0 + j) * 128:(kt0 + j + 1) * 128],
                                             q_t[h][:, sl], start=True, stop=True)
                        et = expp.tile([128, KP * QB], mdt, name=f"e_{h}_{qb}_{kt0}",
                                       tag="et", bufs=12)
                        nc.scalar.activation(et[:], ps[:], Exp, scale=SCALE)
                        for j in range(KP):
                            kt = kt0 + j
                            nc.tensor.matmul(po[:], v_t[:, kt * FKV + f * 128: kt * FKV + (f + 1) * 128],
                                             et[:, j * QB:(j + 1) * QB],
                                             start=(kt == 0), stop=(kt == KT - 1))
                        if KP == 2:
                            node = spool.tile([128, QB], mdt, name=f"pa_{h}_{qb}_{kt0}",
                                              tag=f"pa{(kt0 // 2) % 4}", bufs=3)
                            nc.vector.tensor_add(node[:], et[:, 0:QB], et[:, QB:2 * QB])
                            rank = 1
                        else:
                            node, rank = et, 0
                        while rank in ranks:
                            prev = ranks.pop(rank)
                            nc.vector.tensor_add(prev[:], prev[:], node[:])
                            node, rank = prev, rank + 1
                        ranks[rank] = node
                    rem = [ranks[r] for r in sorted(ranks)]
                    ssum = rem[0]
                    for other in rem[1:]:
                        nc.vector.tensor_add(ssum[:], ssum[:], other[:])
                    pb = psb.tile([128, QB], f32, name=f"pb_{h}_{qb}", tag="bb", bufs=2)
                    nc.tensor.matmul(pb[:], ones_t[:], ssum[:], start=True, stop=True)
                    inv = invp.tile([128, QB], f32, name=f"inv_{h}_{qb}", tag="inv")
                    nc.vector.reciprocal_approx_fast(inv[:], pb[:])
                    # normalized attn output, overwrites q head in place
                    nc.vector.tensor_mul(q_t[h][:, sl], po[:], inv[:])

        # ---- Phase 5: output projection (partial; host sums over TP) ----
        with (
            tc.tile_pool(name="osb", bufs=4) as osb,
            tc.tile_pool(name="po5", bufs=1, space="PSUM") as po5,
        ):
            NHB = HID // 512
            for hb in range(NHB):
                if hb == 0:
                    wo_s = wo_pre
                else:
                    wo_s = []
                    for fh in range(QH):
                        w = st3.tile([128, 512], mdt, name=f"wo_{hb}_{fh}", tag=f"wo{fh}", bufs=2)
                        nc.sync.dma_start(
                            w[:], bc(wo.ap()[fh * 128:(fh + 1) * 128, hb * 512:(hb + 1) * 512]))
                        wo_s.append(w)
                for tt in range(KT):
                    pot = po5.tile([128, 512], f32, name=f"pot_{hb}_{tt}", tag="po", bufs=4)
                    for fh in range(QH):
                        nc.tensor.matmul(pot[:], q_t[fh][:, tt * 128:(tt + 1) * 128],
                                         wo_s[fh][:], start=(fh == 0), stop=(fh == QH - 1))
                    ot = osb.tile([128, 512], f32, name=f"ot_{hb}_{tt}", tag="ot")
                    nc.scalar.copy(ot[:], pot[:])
                    nc.sync.dma_start(
                        out.ap()[tt * 128:(tt + 1) * 128, hb * 512:(hb + 1) * 512], ot[:])

    nc.compile()
    return nc


def make_host_constants():
    ident = np.eye(128, dtype=np.float32)
    ones = np.ones((128, 128), dtype=np.float32)
    return ident, ones


def shard_inputs(hidden_states, cos, sin, Wq, bq, Wk, bk, Wv, bv, Wo, S=S_FULL,
                 dt="bf16"):
    ident, ones = make_host_constants()
    if dt == "bf16":
        import ml_dtypes
        big = ml_dtypes.bfloat16
    else:
        big = np.float32
    in_maps = []
    for c in range(8):
        b, t = c // TP, c % TP
        sinT = np.ascontiguousarray(sin[b].T).astype(np.float32)
        sinT[:HD // 2, :] *= -1.0   # rotate_half sign folded into the table
        m = {
            "hsT": np.ascontiguousarray(hidden_states[b].T).astype(big),
            "cosT": np.ascontiguousarray(cos[b].T).astype(big),
            "sinT": sinT.astype(big),
            "wq": np.ascontiguousarray(Wq[:, t * FQ:(t + 1) * FQ]).astype(big),
            "bq": np.ascontiguousarray(bq[t * FQ:(t + 1) * FQ].reshape(QH, HD)),
            "bk": np.ascontiguousarray(bk[t * FKV:(t + 1) * FKV].reshape(KVH, HD)),
            "bv": np.ascontiguousarray(bv[t * FKV:(t + 1) * FKV].reshape(KVH, HD)),
            "wkv": np.ascontiguousarray(np.concatenate(
                [Wk[:, t * FKV:(t + 1) * FKV], Wv[:, t * FKV:(t + 1) * FKV]],
                axis=1)).astype(big),
            "wo": np.ascontiguousarray(Wo[t * FQ:(t + 1) * FQ, :]).astype(big),
            "ident": ident.astype(big), "ones": ones.astype(big),
        }
        in_maps.append(m)
    return in_maps


_nc_cache = {}


def kernel(hidden_states, cos, sin, Wq, bq, Wk, bk, Wv, bv, Wo):
    global last_exec_time_ns
    from concourse.bass_utils import run_bass_kernel_spmd

    hidden_states = np.asarray(hidden_states, dtype=np.float32)
    cos = np.asarray(cos, dtype=np.float32)
    sin = np.asarray(sin, dtype=np.float32)
    S = hidden_states.shape[1]
    dt = os.environ.get("ATTN_DT", "bf16")
    if (S, dt) not in _nc_cache:
        _nc_cache[(S, dt)] = build_nc(S, dt)
    nc = _nc_cache[(S, dt)]
    in_maps = shard_inputs(hidden_states, cos, sin,
                           np.asarray(Wq, np.float32), np.asarray(bq, np.float32),
                           np.asarray(Wk, np.float32), np.asarray(bk, np.float32),
                           np.asarray(Wv, np.float32), np.asarray(bv, np.float32),
                           np.asarray(Wo, np.float32), S=S, dt=dt)
    trace = bool(int(os.environ.get("ATTN_TRACE", "0")))
    r = run_bass_kernel_spmd(nc, in_maps, list(range(8)), trace=trace)
    last_exec_time_ns = r.exec_time_ns
    outs = [r.results[c]["out"] for c in range(8)]
    full = np.empty((B, S, HID), dtype=np.float32)
    for b in range(B):
        full[b] = outs[b * TP]
        for t in range(1, TP):
            full[b] += outs[b * TP + t]
    return full
